# revision 12
# baseline (speedup 1.0000x reference)
"""Distributed Trainium2 kernel for DeepseekV3-style GQA attention.

Problem (hardcoded): B=1, S=4096, H=2048, NQ=16 q heads, NKV=4 kv heads,
D=128, rotate-half RoPE (theta=10000) over full head dim, causal softmax,
o_proj. 8 NeuronCores, tensor-parallel over heads:

  core c: q heads {2c, 2c+1}, kv head c//2 (replicated across the pair),
  Wq/Wk/Wv column-sharded, attention computed flash-style in bf16 with
  f32 PSUM accumulation, AllGathered in 4 sequence chunks via AllToAll,
  o_proj row-blocks [128, H] per chunk interleaved into the attention
  loop as PE filler work.

v2 changes vs v1:
  - flipped PV: V is the stationary operand, pt (probs) streams 512 wide
    per head, so the attention output lands directly in the transposed
    [d, q] layout the A2A needs -- the 64 XBAR transpose DMAs (79us of
    sync-queue time) and the per-qsub norm dance are gone, and PV drops
    from 1056 N=129 matmuls to 288 N=512 ones
  - softmax denominators: pt tiles are summed on the (idle) Vector engine
    into a per-supertile bf16 accumulator; one M=1 ones-matmul per head
    reduces it across partitions, reciprocal + DMA row-broadcast feed a
    single tensor_tensor normalize per head
  - DMA traffic spread across rings: xt chunks alternate vector/sync,
    weights+cos/sin+wo ride scalar, so no single ring serializes
  - fine-grained startup: first matmul gated on 0.5MB of wq + 1MB of xt
    instead of 5MB on one ring
  - o_proj chunks enqueued earlier (qs=4/5/6) with hold for the A2A(3) gap
"""
import os
import sys

sys.path.insert(0, "/opt/trn_rl_repo")

import numpy as np
import ml_dtypes

import concourse.bass as bass
import concourse.bacc as bacc
import concourse.mybir as mybir
import concourse.tile as tile
from concourse.bass_utils import run_bass_kernel_spmd

BF16 = mybir.dt.bfloat16
F32 = mybir.dt.float32
NPBF16 = ml_dtypes.bfloat16

B, S, H = 1, 4096, 2048
NQ, NKV, D = 16, 4, 128
THETA = 10000.0
NCORES = 8
HPC = NQ // NCORES          # q heads per core = 2
SC = 512                    # projection s-chunk
NSC = S // SC               # 8
NKT = S // 128              # 32 k tiles of 128
QS = 512                    # attention q supertile
NQS = S // QS               # 8
CHUNK = 1024                # allgather s-chunk
NCH = S // CHUNK            # 4
SCALE = 1.0 / float(np.sqrt(D))

_cached = {}


def _build():
    nc = bacc.Bacc("TRN2", target_bir_lowering=False, debug=False,
                   num_devices=NCORES)

    xT = nc.declare_dram_parameter("xT", [NSC, 128, 16 * SC], BF16, isOutput=False)
    wq = nc.declare_dram_parameter("wq", [128, 16 * HPC * D], BF16, isOutput=False)
    wk = nc.declare_dram_parameter("wk", [128, 16 * D], BF16, isOutput=False)
    wv = nc.declare_dram_parameter("wv", [128, 16 * D], BF16, isOutput=False)
    wo = nc.declare_dram_parameter("wo", [128, 16 * H], BF16, isOutput=False)
    cosT = nc.declare_dram_parameter("cosT", [D, S], BF16, isOutput=False)
    sinT = nc.declare_dram_parameter("sinT", [D, S], BF16, isOutput=False)
    masksl = nc.declare_dram_parameter("masksl", [128, 128], BF16, isOutput=False)
    identity = nc.declare_dram_parameter("identity", [128, 128], BF16, isOutput=False)
    out = nc.declare_dram_parameter("out", [NCH * 128, H], BF16, isOutput=True)

    with tile.TileContext(nc) as tc:
        with (
            tc.tile_pool(name="const", bufs=1) as constp,
            tc.tile_pool(name="persist", bufs=1) as persist,
            tc.tile_pool(name="xtp", bufs=2) as xtp,
            tc.tile_pool(name="ropep", bufs=2) as ropep,
            tc.tile_pool(name="ptp", bufs=3) as ptp,
            tc.tile_pool(name="accp", bufs=2) as accp,
            tc.tile_pool(name="attnp", bufs=2) as attnp,
            tc.tile_pool(name="smallp", bufs=2) as smallp,
            tc.tile_pool(name="rbp", bufs=2) as rbp,
            tc.tile_pool(name="agp", bufs=2) as agp,
            tc.tile_pool(name="outp", bufs=1) as outp,
            tc.tile_pool(name="dram", bufs=1, space="DRAM") as dramp,
        ):
            wq_all = persist.tile([128, 16 * HPC * D], BF16, tag="wq")
            wk_all = persist.tile([128, 16 * D], BF16, tag="wk")
            wv_all = persist.tile([128, 16 * D], BF16, tag="wv")
            cos_sb = persist.tile([128, S], BF16, tag="cos")
            sin_sb = persist.tile([128, S], BF16, tag="sin")
            msl = constp.tile([128, 128], BF16, tag="msl")
            ident = constp.tile([128, 128], BF16, tag="ident")
            ones_t = constp.tile([128, 1], BF16, tag="ones")
            ones_r = constp.tile([1, 128], BF16, tag="ones_r")
            wo_all = persist.tile([128, 16 * H], BF16, tag="wo")
            nc.gpsimd.memset(ones_t[:], 1.0)
            nc.gpsimd.memset(ones_r[:], 1.0)

            def wqt(t, h):
                return wq_all[:, t * HPC * D + h * D: t * HPC * D + (h + 1) * D]

            QT = [persist.tile([128, S], BF16, tag=f"qt{h}", name=f"qt{h}")
                  for h in range(HPC)]
            KT = persist.tile([128, S], BF16, tag="kt")
            V = [persist.tile([128, D], BF16, tag=f"v{kt}", name=f"v{kt}")
                 for kt in range(NKT)]

            # tiny barrier collective: warms the CC stream during the
            # input-DMA/proj phase so the first real A2A doesn't pay it
            bar_in = dramp.tile([1, 128], F32, tag="bar_in", name="bar_in")
            bar_out = dramp.tile([1, 128], F32, tag="bar_out", name="bar_out",
                                 addr_space="Shared")
            nc.gpsimd.collective_compute(
                "AllReduce", mybir.AluOpType.add,
                replica_groups=[list(range(NCORES))],
                ins=[bar_in.opt()], outs=[bar_out.opt()])

            # startup DMAs: the first matmul's waits coalesce to ALL DMAs
            # issued before it per-ring, so keep each ring's prefix to just
            # what chunk 0 needs first (wq_p0 on sync; xt0 piece 0 is the
            # first scalar-ring DMA, issued in the proj loop below). Only
            # sync (SP) and scalar (Act) have HWDGE rings on this config.
            nc.sync.dma_start(wq_all[:, 0:2048], wq[:, 0:2048])

            # ---- phase 1: projections (chunked over s) + RoPE ----
            with (
                tc.tile_pool(name="projps", bufs=4, space="PSUM") as projps,
                tc.tile_pool(name="vps", bufs=2, space="PSUM") as vps,
            ):
                for sc in range(NSC):
                    off = sc * SC
                    xt_all = xtp.tile([128, 16 * SC], BF16, tag="xt",
                                      name="xt_all")
                    ring = nc.scalar if sc % 2 == 0 else nc.sync
                    ring.dma_start(xt_all[:, 0:4096], xT[sc][:, 0:4096])
                    if sc == 0:
                        # interleave the early-needed weights/tables into
                        # the scalar ring right after xt0's first piece
                        nc.scalar.dma_start(wk_all[:], wk[:])
                        nc.scalar.dma_start(cos_sb[:, 0:2048],
                                            cosT[:, 0:2048])
                    ring.dma_start(xt_all[:, 4096:8192], xT[sc][:, 4096:8192])
                    if sc == 0:
                        nc.scalar.dma_start(sin_sb[:, 0:2048],
                                            sinT[:, 0:2048])
                        nc.scalar.dma_start(wv_all[:], wv[:])
                        nc.scalar.dma_start(msl[:], masksl[:])
                        nc.scalar.dma_start(ident[:], identity[:])
                        nc.scalar.dma_start(cos_sb[:, 2048:S],
                                            cosT[:, 2048:S])
                        nc.scalar.dma_start(sin_sb[:, 2048:S],
                                            sinT[:, 2048:S])

                    def xts(t):
                        return xt_all[:, t * SC:(t + 1) * SC]

                    # q/k projections -> transposed layout [d, s]
                    def project_rope(lhs_of_t, dst, split_after=None):
                        ps = projps.tile([128, SC], F32, tag="proj", name="ps")
                        for t in range(16):
                            nc.tensor.matmul(ps[:], lhs_of_t(t), xts(t),
                                             start=(t == 0), stop=(t == 15))
                            if split_after is not None and t == split_after:
                                # late-issue the rest of wq so the first
                                # matmuls only gate on the first half
                                nc.sync.dma_start(wq_all[:, 2048:4096],
                                                  wq[:, 2048:4096])
                        raw = ropep.tile([128, SC], BF16, tag="raw", name="raw")
                        nc.vector.tensor_copy(raw[:], ps[:])
                        sw = ropep.tile([128, SC], BF16, tag="sw", name="sw")
                        nc.sync.dma_start(sw[0:64, :], raw[64:128, :])
                        nc.sync.dma_start(sw[64:128, :], raw[0:64, :])
                        t1 = ropep.tile([128, SC], BF16, tag="t1", name="t1")
                        nc.vector.tensor_tensor(t1[:], raw[:],
                                                cos_sb[:, off:off + SC],
                                                mybir.AluOpType.mult)
                        t2 = ropep.tile([128, SC], BF16, tag="t2", name="t2")
                        nc.vector.tensor_tensor(t2[:], sw[:],
                                                sin_sb[:, off:off + SC],
                                                mybir.AluOpType.mult)
                        nc.vector.tensor_tensor(dst[:, off:off + SC], t1[:],
                                                t2[:], mybir.AluOpType.add)

                    for h in range(HPC):
                        project_rope(lambda t, h=h: wqt(t, h), QT[h],
                                     split_after=7 if (sc == 0 and h == 0)
                                     else None)
                    project_rope(lambda t: wk_all[:, t * D:(t + 1) * D], KT)

                    # v projection (natural [s, d] layout)
                    for st in range(SC // 128):
                        v_ps = vps.tile([128, 128], F32, tag="vps", name="v_ps")
                        for t in range(16):
                            nc.tensor.matmul(
                                v_ps[:], xt_all[:, t * SC + st * 128:
                                                t * SC + (st + 1) * 128],
                                wv_all[:, t * D:(t + 1) * D],
                                start=(t == 0), stop=(t == 15))
                        kti = sc * (SC // 128) + st
                        nc.vector.tensor_copy(V[kti][:], v_ps[:])

            # wo (8.4MB) deliberately loads during attention, not proj:
            # the proj phase is nearly DMA-bound (xt stream), attention
            # DMA is light, and wo is first read at the qs=4 o_proj fill
            nc.scalar.dma_start(wo_all[:], wo[:])

            # ---- phases 2+3 interleaved: attention, AG, o_proj ----
            with (
                tc.tile_pool(name="stps", bufs=2, space="PSUM") as stps,
                tc.tile_pool(name="attps", bufs=1, space="PSUM") as attps,
                tc.tile_pool(name="ops", bufs=1, space="PSUM") as opsp,
                tc.tile_pool(name="dnps", bufs=1, space="PSUM") as dnps,
            ):
                attnT_cur = [None, None]
                bounces = []
                fillers = []        # pending o_proj closures (PE work units)
                hold = [0]          # units to keep queued for the tail gap

                def pump(n):
                    for _ in range(n):
                        if len(fillers) <= hold[0]:
                            return
                        fillers.pop(0)()

                def attention_pair(qs):
                    """Both heads per kt: paired score psum [h0|h1] (2
                    banks), one 2N-wide exp, then a single V-stationary
                    matmul per head streams pt 512 wide into a [d, q]
                    accumulator -- output is born transposed. Softmax
                    denominators ride a bf16 DVE accumulator + one M=1
                    ones-matmul per head at supertile end."""
                    q_off = qs * QS
                    nkt = 4 * qs + 4
                    att = [attps.tile([128, QS], F32, tag=f"att{h}",
                                      name=f"att{h}_{qs}")
                           for h in range(HPC)]
                    acc = accp.tile([128, 2 * QS], BF16, tag="acc",
                                    name=f"acc{qs}")

                    def _pv(kt, pt):
                        for h in range(HPC):
                            nc.tensor.matmul(
                                att[h][:], V[kt][:],
                                pt[:, h * QS:(h + 1) * QS],
                                start=(kt == 0), stop=(kt == nkt - 1),
                                skip_group_check=True)

                    pend = None
                    for kt in range(nkt):
                        j = kt - 4 * qs
                        q_lo = 128 * j if j > 0 else 0
                        sp = stps.tile([128, 2 * QS], F32, tag="st",
                                       name="st_pair")
                        for h in range(HPC):
                            nc.tensor.matmul(
                                sp[:, h * QS + q_lo:(h + 1) * QS],
                                KT[:, kt * 128:(kt + 1) * 128],
                                QT[h][:, q_off + q_lo:q_off + QS],
                                start=True, stop=(j < 0),
                                skip_group_check=True)
                            if j >= 0:
                                # additive causal mask on the diagonal block
                                nc.tensor.matmul(
                                    sp[:, h * QS + q_lo:h * QS + q_lo + 128],
                                    ident[:], msl[:], start=False, stop=True,
                                    skip_group_check=True)
                        pt = ptp.tile([128, 2 * QS], BF16, tag="pt", name="pt")
                        if q_lo > 0:
                            # columns below the diagonal band are fully
                            # masked; zero them so PV and the denominator
                            # accumulate see exact zeros
                            nc.vector.memset(
                                pt[:].rearrange("p (h c) -> p h c", h=2)
                                [:, :, 0:q_lo], 0.0)
                        src = sp[:].rearrange("p (h c) -> p h c",
                                              h=2)[:, :, q_lo:QS]
                        dst = pt[:].rearrange("p (h c) -> p h c",
                                              h=2)[:, :, q_lo:QS]
                        nc.scalar.activation(dst, src,
                                             mybir.ActivationFunctionType.Exp,
                                             scale=SCALE)
                        if kt == 0:
                            nc.vector.tensor_copy(acc[:], pt[:])
                        else:
                            nc.vector.tensor_tensor(acc[:], acc[:], pt[:],
                                                    mybir.AluOpType.add)
                        pump(2)
                        if pend is not None:
                            _pv(*pend)
                        pend = (kt, pt)
                    pump(2)
                    _pv(*pend)

                    # denominators + normalize into the A2A layout.
                    # dn_t's bank is reused: row 0 takes the ones-matmul
                    # column sums, then a K=1 ones-row matmul broadcasts
                    # the reciprocal back over all 128 partitions.
                    col = (qs % 2) * QS
                    for h in range(HPC):
                        dn_t = dnps.tile([128, QS], F32, tag="dn", name="dn_t")
                        nc.tensor.matmul(dn_t[0:1, :], ones_t[:],
                                         acc[:, h * QS:(h + 1) * QS],
                                         start=True, stop=True,
                                         skip_group_check=True)
                        rec = smallp.tile([1, QS], BF16, tag="rec", name="rec")
                        with nc.allow_low_precision(
                                reason="bf16 softmax recip, tol 2e-2"):
                            nc.vector.reciprocal(rec[:], dn_t[0:1, :])
                        nc.tensor.matmul(dn_t[:], ones_r[:], rec[:],
                                         start=True, stop=True,
                                         skip_group_check=True)
                        rb = rbp.tile([128, QS], F32, tag="rb", name="rb")
                        nc.vector.tensor_copy(rb[:], dn_t[:])
                        pump(2)
                        nc.vector.tensor_tensor(
                            attnT_cur[h][:, col:col + QS], att[h][:], rb[:],
                            mybir.AluOpType.mult)
                        pump(2)
                    pump(4)

                def emit_a2a(ci, bounce):
                    ex = dramp.tile([NCORES * 2 * 128, 128], BF16,
                                    tag=f"a2a{ci}", name=f"a2a{ci}")
                    nc.gpsimd.collective_compute(
                        "AllToAll", mybir.AluOpType.bypass,
                        replica_groups=[list(range(NCORES))],
                        ins=[bounce.opt()], outs=[ex.opt()])
                    bounces.append(ex)

                def enqueue_oproj(ci):
                    """Queue chunk ci's o_proj as filler closures."""
                    ex = bounces[ci]
                    ag_all = agp.tile([128, 16 * 128], BF16, tag="ag",
                                      name="ag_all")
                    nc.sync.dma_start(
                        ag_all[:].rearrange("p (t s) -> p t s", t=16),
                        ex[:].rearrange("(t p) s -> p t s", p=128))
                    o_sb = outp.tile([128, H], BF16, tag="osb", name="o_sb")
                    state = {}

                    def mk_mm(ocg, jt):
                        def run():
                            if jt == 0:
                                state[ocg] = opsp.tile([128, 512], F32,
                                                       tag="ops", name="o_ps")
                            nc.tensor.matmul(
                                state[ocg][:],
                                ag_all[:, jt * 128:(jt + 1) * 128],
                                wo_all[:, jt * H + ocg * 512:
                                       jt * H + (ocg + 1) * 512],
                                start=(jt == 0), stop=(jt == 15))
                        return run

                    def mk_copy(ocg):
                        def run():
                            nc.vector.tensor_copy(
                                o_sb[:, ocg * 512:(ocg + 1) * 512],
                                state[ocg][:])
                        return run

                    def mk_store(ocg):
                        def run():
                            nc.sync.dma_start(
                                out[ci * 128:(ci + 1) * 128,
                                    ocg * 512:(ocg + 1) * 512],
                                o_sb[:, ocg * 512:(ocg + 1) * 512])
                        return run

                    for ocg in range(H // 512):
                        for jt in range(16):
                            fillers.append(mk_mm(ocg, jt))
                        fillers.append(mk_copy(ocg))
                        fillers.append(mk_store(ocg))

                bounce_cur = None
                for qs in range(NQS):
                    ci = qs // 2
                    if qs % 2 == 0:
                        bounce_cur = dramp.tile([NCORES * 2 * 128, 128], BF16,
                                                tag=f"bn{ci}", name=f"bn{ci}")
                        for head in range(HPC):
                            attnT_cur[head] = attnp.tile(
                                [128, CHUNK], BF16, tag=f"attnT{head}",
                                name=f"attnT{head}_{qs}")
                    # consume A2As a couple of supertiles after emission so
                    # each one has fully landed before its o_proj pumps
                    if qs == 4:
                        enqueue_oproj(0)
                    elif qs == 5:
                        enqueue_oproj(1)
                    elif qs == 6:
                        enqueue_oproj(2)
                        hold[0] = 60   # keep PE work for the A2A(3) gap
                    attention_pair(qs)
                    if qs % 2 == 1:
                        # scatter this chunk's attnT into the A2A bounce:
                        # dest core d gets s cols d*128..(d+1)*128 of the chunk
                        bv = bounce_cur[:].rearrange(
                            "(d h j) s -> h j d s", d=NCORES, h=HPC)
                        for head in range(HPC):
                            nc.sync.dma_start(
                                bv[head],
                                attnT_cur[head][:].rearrange(
                                    "j (d s) -> j d s", d=NCORES))
                        emit_a2a(ci, bounce_cur)
                # tail: the held-back chunk-2 units cover the A2A(3) wait
                hold[0] = 0
                pump(len(fillers))
                enqueue_oproj(3)
                pump(len(fillers))

    nc.compile()
    return nc


def _get_nc():
    if "nc" not in _cached:
        _cached["nc"] = _build()
    return _cached["nc"]


def _prep_inputs(hidden_states, Wq, Wk, Wv, Wo, position_ids):
    x = np.asarray(hidden_states, dtype=np.float32).reshape(S, H)
    xT = np.ascontiguousarray(
        x.T.reshape(16, 128, NSC, SC).transpose(2, 1, 0, 3)
        .reshape(NSC, 128, 16 * SC)).astype(NPBF16)

    def wshuf(W):
        n = W.shape[1]
        return np.ascontiguousarray(
            W.reshape(16, 128, n).transpose(1, 0, 2).reshape(128, 16 * n)
        ).astype(NPBF16)
    Wq = np.asarray(Wq, dtype=np.float32)
    Wk = np.asarray(Wk, dtype=np.float32)
    Wv = np.asarray(Wv, dtype=np.float32)
    Wo = np.asarray(Wo, dtype=np.float32)
    pos = np.asarray(position_ids).reshape(S).astype(np.float32)

    half = D // 2
    inv_freq = 1.0 / (THETA ** (np.arange(half, dtype=np.float32) * 2.0 / D))
    freqs = inv_freq[:, None] * pos[None, :]          # [64, S]
    c64 = np.cos(freqs, dtype=np.float32)
    s64 = np.sin(freqs, dtype=np.float32)
    cosT = np.vstack([c64, c64]).astype(NPBF16)       # [128, S]
    sinT = np.vstack([-s64, s64]).astype(NPBF16)      # signed for rotate-half
    masksl = (np.tril(np.ones((128, 128), dtype=np.float32), -1)
              * -30000.0).astype(NPBF16)
    Wo_bf = wshuf(Wo)
    ident = np.eye(128, dtype=np.float32).astype(NPBF16)

    in_maps = []
    for c in range(NCORES):
        kvh = c // 2
        in_maps.append({
            "xT": xT,
            "wq": wshuf(Wq[:, c * HPC * D:(c + 1) * HPC * D]),
            "wk": wshuf(Wk[:, kvh * D:(kvh + 1) * D]),
            "wv": wshuf(Wv[:, kvh * D:(kvh + 1) * D]),
            "wo": Wo_bf,
            "cosT": cosT,
            "sinT": sinT,
            "masksl": masksl,
            "identity": ident,
        })
    return in_maps


def _run(inputs, trace=False):
    nc = _get_nc()
    in_maps = _prep_inputs(**inputs)
    res = run_bass_kernel_spmd(nc, in_maps, list(range(NCORES)), trace=trace)
    full = np.empty((S, H), dtype=np.float32)
    for c in range(NCORES):
        shard = np.asarray(res.results[c]["out"], dtype=np.float32)
        for i in range(NCH):
            full[i * CHUNK + c * 128: i * CHUNK + (c + 1) * 128, :] = \
                shard[i * 128:(i + 1) * 128, :]
    return full.reshape(B, S, H), res


def kernel(**inputs):
    full, _ = _run(inputs, trace=False)
    return full


# revision 18
# speedup vs baseline: 1.0735x; 1.0735x over previous
"""Distributed Trainium2 kernel for DeepseekV3-style GQA attention.

Problem (hardcoded): B=1, S=4096, H=2048, NQ=16 q heads, NKV=4 kv heads,
D=128, rotate-half RoPE (theta=10000) over full head dim, causal softmax,
o_proj. 8 NeuronCores, tensor-parallel over heads:

  core c: q heads {2c, 2c+1}, kv head c//2 (replicated across the pair),
  Wq/Wk/Wv column-sharded, attention computed flash-style in bf16 with
  f32 PSUM accumulation, AllGathered in 4 sequence chunks via AllToAll,
  o_proj row-blocks [128, H] per chunk interleaved into the attention
  loop as PE filler work.

v2 changes vs v1:
  - flipped PV: V is the stationary operand, pt (probs) streams 512 wide
    per head, so the attention output lands directly in the transposed
    [d, q] layout the A2A needs -- the 64 XBAR transpose DMAs (79us of
    sync-queue time) and the per-qsub norm dance are gone, and PV drops
    from 1056 N=129 matmuls to 288 N=512 ones
  - softmax denominators: pt tiles are summed on the (idle) Vector engine
    into a per-supertile bf16 accumulator; one M=1 ones-matmul per head
    reduces it across partitions, reciprocal + DMA row-broadcast feed a
    single tensor_tensor normalize per head
  - DMA traffic spread across rings: xt chunks alternate vector/sync,
    weights+cos/sin+wo ride scalar, so no single ring serializes
  - fine-grained startup: first matmul gated on 0.5MB of wq + 1MB of xt
    instead of 5MB on one ring
  - o_proj chunks enqueued earlier (qs=4/5/6) with hold for the A2A(3) gap
"""
import os
import sys

sys.path.insert(0, "/opt/trn_rl_repo")

import numpy as np
import ml_dtypes

import concourse.bass as bass
import concourse.bacc as bacc
import concourse.mybir as mybir
import concourse.tile as tile
from concourse.bass_utils import run_bass_kernel_spmd

BF16 = mybir.dt.bfloat16
F32 = mybir.dt.float32
NPBF16 = ml_dtypes.bfloat16

B, S, H = 1, 4096, 2048
NQ, NKV, D = 16, 4, 128
THETA = 10000.0
NCORES = 8
HPC = NQ // NCORES          # q heads per core = 2
SC = 512                    # projection s-chunk
NSC = S // SC               # 8
NKT = S // 128              # 32 k tiles of 128
QS = 512                    # attention q supertile
NQS = S // QS               # 8
CHUNK = 1024                # allgather s-chunk
NCH = S // CHUNK            # 4
SCALE = 1.0 / float(np.sqrt(D))

_cached = {}


def _build():
    nc = bacc.Bacc("TRN2", target_bir_lowering=False, debug=False,
                   num_devices=NCORES)

    xT = nc.declare_dram_parameter("xT", [NSC, 128, 16 * SC], BF16, isOutput=False)
    wq = nc.declare_dram_parameter("wq", [128, 16 * HPC * D], BF16, isOutput=False)
    wk = nc.declare_dram_parameter("wk", [128, 16 * D], BF16, isOutput=False)
    wv = nc.declare_dram_parameter("wv", [128, 16 * D], BF16, isOutput=False)
    wo = nc.declare_dram_parameter("wo", [128, 16 * H], BF16, isOutput=False)
    cosT = nc.declare_dram_parameter("cosT", [D, S], BF16, isOutput=False)
    sinT = nc.declare_dram_parameter("sinT", [D, S], BF16, isOutput=False)
    masksl = nc.declare_dram_parameter("masksl", [128, 128], BF16, isOutput=False)
    identity = nc.declare_dram_parameter("identity", [128, 128], BF16, isOutput=False)
    out = nc.declare_dram_parameter("out", [NCH * 128, H], BF16, isOutput=True)

    with tile.TileContext(nc) as tc:
        with (
            tc.tile_pool(name="const", bufs=1) as constp,
            tc.tile_pool(name="persist", bufs=1) as persist,
            tc.tile_pool(name="xtp", bufs=3) as xtp,
            tc.tile_pool(name="ropep", bufs=3) as ropep,
            tc.tile_pool(name="ptp", bufs=4) as ptp,
            tc.tile_pool(name="accp", bufs=2) as accp,
            tc.tile_pool(name="attnp", bufs=2) as attnp,
            tc.tile_pool(name="smallp", bufs=2) as smallp,
            tc.tile_pool(name="rbp", bufs=2) as rbp,
            tc.tile_pool(name="agp", bufs=2) as agp,
            tc.tile_pool(name="outp", bufs=1) as outp,
            tc.tile_pool(name="dram", bufs=1, space="DRAM") as dramp,
        ):
            wq_all = persist.tile([128, 16 * HPC * D], BF16, tag="wq")
            wk_all = persist.tile([128, 16 * D], BF16, tag="wk")
            wv_all = persist.tile([128, 16 * D], BF16, tag="wv")
            cos_sb = persist.tile([128, S], BF16, tag="cos")
            sin_sb = persist.tile([128, S], BF16, tag="sin")
            msl = constp.tile([128, 128], BF16, tag="msl")
            ident = constp.tile([128, 128], BF16, tag="ident")
            ones_t = constp.tile([128, 1], BF16, tag="ones")
            ones_r = constp.tile([1, 128], BF16, tag="ones_r")
            wo_all = persist.tile([128, 16 * H], BF16, tag="wo")
            nc.gpsimd.memset(ones_t[:], 1.0)
            nc.gpsimd.memset(ones_r[:], 1.0)

            def wqt(t, h):
                return wq_all[:, t * HPC * D + h * D: t * HPC * D + (h + 1) * D]

            QT = [persist.tile([128, S], BF16, tag=f"qt{h}", name=f"qt{h}")
                  for h in range(HPC)]
            KT = persist.tile([128, S], BF16, tag="kt")
            V = [persist.tile([128, D], BF16, tag=f"v{kt}", name=f"v{kt}")
                 for kt in range(NKT)]

            # tiny barrier collective: warms the CC stream during the
            # input-DMA/proj phase so the first real A2A doesn't pay it
            bar_in = dramp.tile([1, 128], F32, tag="bar_in", name="bar_in")
            bar_out = dramp.tile([1, 128], F32, tag="bar_out", name="bar_out",
                                 addr_space="Shared")
            nc.gpsimd.collective_compute(
                "AllReduce", mybir.AluOpType.add,
                replica_groups=[list(range(NCORES))],
                ins=[bar_in.opt()], outs=[bar_out.opt()])

            # startup DMAs: the first matmul's waits coalesce to ALL DMAs
            # issued before it per-ring, so keep each ring's prefix to just
            # what chunk 0 needs first (tiny wq piece on sync; xt0's first
            # t-block is the first scalar-ring DMA, issued in the proj loop
            # below). Only sync (SP) and scalar (Act) have HWDGE rings.
            nc.sync.dma_start(wq_all[:, 0:256], wq[:, 0:256])
            nc.sync.dma_start(wq_all[:, 256:2048], wq[:, 256:2048])

            # ---- phase 1: projections (chunked over s) + RoPE ----
            with (
                tc.tile_pool(name="projps", bufs=4, space="PSUM") as projps,
                tc.tile_pool(name="vps", bufs=2, space="PSUM") as vps,
            ):
                for sc in range(NSC):
                    off = sc * SC
                    ring = nc.scalar if sc % 2 == 0 else nc.sync
                    xt_a = xtp.tile([128, 8 * SC], BF16, tag="xt",
                                    name="xt_a")
                    if sc == 0:
                        # first t-block alone so the first matmul gates on
                        # only 128KB of x
                        nc.scalar.dma_start(xt_a[:, 0:SC], xT[0][:, 0:SC])
                        nc.scalar.dma_start(xt_a[:, SC:4096],
                                            xT[0][:, SC:4096])
                        # interleave the early-needed weights/tables into
                        # the scalar ring right after xt0's pieces
                        nc.scalar.dma_start(wk_all[:], wk[:])
                        nc.scalar.dma_start(cos_sb[:, 0:2048],
                                            cosT[:, 0:2048])
                    else:
                        ring.dma_start(xt_a[:], xT[sc][:, 0:4096])
                    xt_b = xtp.tile([128, 8 * SC], BF16, tag="xt",
                                    name="xt_b")
                    ring.dma_start(xt_b[:], xT[sc][:, 4096:8192])
                    if sc == 0:
                        nc.scalar.dma_start(sin_sb[:, 0:2048],
                                            sinT[:, 0:2048])
                        nc.scalar.dma_start(wv_all[:], wv[:])
                        nc.scalar.dma_start(msl[:], masksl[:])
                        nc.scalar.dma_start(ident[:], identity[:])
                    elif sc == 2:
                        # second halves of the rope tables: first needed by
                        # chunk 4, so issue them behind chunk 2's x stream
                        nc.scalar.dma_start(cos_sb[:, 2048:S],
                                            cosT[:, 2048:S])
                        nc.scalar.dma_start(sin_sb[:, 2048:S],
                                            sinT[:, 2048:S])

                    def xts(t):
                        half = xt_a if t < 8 else xt_b
                        return half[:, (t % 8) * SC:(t % 8 + 1) * SC]

                    # q/k projections -> transposed layout [d, s]
                    def project_rope(lhs_of_t, dst, split_after=None):
                        ps = projps.tile([128, SC], F32, tag="proj", name="ps")
                        for t in range(16):
                            nc.tensor.matmul(ps[:], lhs_of_t(t), xts(t),
                                             start=(t == 0), stop=(t == 15))
                            if split_after is not None and t == split_after:
                                # late-issue the rest of wq so the first
                                # matmuls only gate on the first half
                                nc.sync.dma_start(wq_all[:, 2048:4096],
                                                  wq[:, 2048:4096])
                        raw = ropep.tile([128, SC], BF16, tag="raw", name="raw")
                        nc.vector.tensor_copy(raw[:], ps[:])
                        sw = ropep.tile([128, SC], BF16, tag="sw", name="sw")
                        nc.sync.dma_start(sw[0:64, :], raw[64:128, :])
                        nc.sync.dma_start(sw[64:128, :], raw[0:64, :])
                        t1 = ropep.tile([128, SC], BF16, tag="t1", name="t1")
                        nc.vector.tensor_tensor(t1[:], raw[:],
                                                cos_sb[:, off:off + SC],
                                                mybir.AluOpType.mult)
                        t2 = ropep.tile([128, SC], BF16, tag="t2", name="t2")
                        nc.vector.tensor_tensor(t2[:], sw[:],
                                                sin_sb[:, off:off + SC],
                                                mybir.AluOpType.mult)
                        nc.vector.tensor_tensor(dst[:, off:off + SC], t1[:],
                                                t2[:], mybir.AluOpType.add)

                    for h in range(HPC):
                        project_rope(lambda t, h=h: wqt(t, h), QT[h],
                                     split_after=7 if (sc == 0 and h == 0)
                                     else None)
                    project_rope(lambda t: wk_all[:, t * D:(t + 1) * D], KT)

                    # v projection (natural [s, d] layout)
                    for st in range(SC // 128):
                        v_ps = vps.tile([128, 128], F32, tag="vps", name="v_ps")
                        for t in range(16):
                            half = xt_a if t < 8 else xt_b
                            nc.tensor.matmul(
                                v_ps[:], half[:, (t % 8) * SC + st * 128:
                                              (t % 8) * SC + (st + 1) * 128],
                                wv_all[:, t * D:(t + 1) * D],
                                start=(t == 0), stop=(t == 15))
                        kti = sc * (SC // 128) + st
                        nc.vector.tensor_copy(V[kti][:], v_ps[:])

            # wo (8.4MB) deliberately loads during attention, not proj:
            # the proj phase is nearly DMA-bound (xt stream), attention
            # DMA is light, and wo is first read at the qs=4 o_proj fill
            nc.scalar.dma_start(wo_all[:], wo[:])

            # ---- phases 2+3 interleaved: attention, AG, o_proj ----
            with (
                tc.tile_pool(name="stps", bufs=2, space="PSUM") as stps,
                tc.tile_pool(name="attps", bufs=1, space="PSUM") as attps,
                tc.tile_pool(name="ops", bufs=1, space="PSUM") as opsp,
                tc.tile_pool(name="dnps", bufs=1, space="PSUM") as dnps,
            ):
                attnT_cur = [None, None]
                bounces = []
                fillers = []        # pending o_proj closures (PE work units)
                hold = [0]          # units to keep queued for the tail gap

                def pump(n):
                    for _ in range(n):
                        if len(fillers) <= hold[0]:
                            return
                        fillers.pop(0)()

                def attention_pair(qs):
                    """Both heads per kt: paired score psum [h0|h1] (2
                    banks), one 2N-wide exp, then a single V-stationary
                    matmul per head streams pt 512 wide into a [d, q]
                    accumulator -- output is born transposed. Softmax
                    denominators ride a bf16 DVE accumulator + one M=1
                    ones-matmul per head at supertile end."""
                    q_off = qs * QS
                    nkt = 4 * qs + 4
                    att = [attps.tile([128, QS], F32, tag=f"att{h}",
                                      name=f"att{h}_{qs}")
                           for h in range(HPC)]
                    acc = accp.tile([128, 2 * QS], BF16, tag="acc",
                                    name=f"acc{qs}")

                    def _pv(kt, pt):
                        for h in range(HPC):
                            nc.tensor.matmul(
                                att[h][:], V[kt][:],
                                pt[:, h * QS:(h + 1) * QS],
                                start=(kt == 0), stop=(kt == nkt - 1),
                                skip_group_check=True)

                    pend = None
                    for kt in range(nkt):
                        j = kt - 4 * qs
                        q_lo = 128 * j if j > 0 else 0
                        sp = stps.tile([128, 2 * QS], F32, tag="st",
                                       name="st_pair")
                        for h in range(HPC):
                            nc.tensor.matmul(
                                sp[:, h * QS + q_lo:(h + 1) * QS],
                                KT[:, kt * 128:(kt + 1) * 128],
                                QT[h][:, q_off + q_lo:q_off + QS],
                                start=True, stop=(j < 0),
                                skip_group_check=True)
                            if j >= 0:
                                # additive causal mask on the diagonal block
                                nc.tensor.matmul(
                                    sp[:, h * QS + q_lo:h * QS + q_lo + 128],
                                    ident[:], msl[:], start=False, stop=True,
                                    skip_group_check=True)
                        pt = ptp.tile([128, 2 * QS], BF16, tag="pt", name="pt")
                        if q_lo > 0:
                            # columns below the diagonal band are fully
                            # masked; zero them so PV and the denominator
                            # accumulate see exact zeros
                            nc.vector.memset(
                                pt[:].rearrange("p (h c) -> p h c", h=2)
                                [:, :, 0:q_lo], 0.0)
                        src = sp[:].rearrange("p (h c) -> p h c",
                                              h=2)[:, :, q_lo:QS]
                        dst = pt[:].rearrange("p (h c) -> p h c",
                                              h=2)[:, :, q_lo:QS]
                        nc.scalar.activation(dst, src,
                                             mybir.ActivationFunctionType.Exp,
                                             scale=SCALE)
                        if kt == 0:
                            nc.vector.tensor_copy(acc[:], pt[:])
                        else:
                            nc.vector.tensor_tensor(acc[:], acc[:], pt[:],
                                                    mybir.AluOpType.add)
                        pump(2)
                        if pend is not None:
                            _pv(*pend)
                        pend = (kt, pt)
                    pump(2)
                    _pv(*pend)

                    # denominators + normalize into the A2A layout.
                    # dn_t's bank is reused: row 0 takes the ones-matmul
                    # column sums, then a K=1 ones-row matmul broadcasts
                    # the reciprocal back over all 128 partitions.
                    col = (qs % 2) * QS
                    for h in range(HPC):
                        dn_t = dnps.tile([128, QS], F32, tag="dn", name="dn_t")
                        nc.tensor.matmul(dn_t[0:1, :], ones_t[:],
                                         acc[:, h * QS:(h + 1) * QS],
                                         start=True, stop=True,
                                         skip_group_check=True)
                        rec_f = smallp.tile([1, QS], F32, tag="rec_f",
                                            name="rec_f")
                        nc.vector.reciprocal_approx_fast(rec_f[:],
                                                         dn_t[0:1, :])
                        rec = smallp.tile([1, QS], BF16, tag="rec", name="rec")
                        nc.vector.tensor_copy(rec[:], rec_f[:])
                        nc.tensor.matmul(dn_t[:], ones_r[:], rec[:],
                                         start=True, stop=True,
                                         skip_group_check=True)
                        rb = rbp.tile([128, QS], BF16, tag="rb", name="rb")
                        nc.vector.tensor_copy(rb[:], dn_t[:])
                        pump(2)
                        nc.vector.tensor_tensor(
                            attnT_cur[h][:, col:col + QS], att[h][:], rb[:],
                            mybir.AluOpType.mult)
                        pump(2)
                    pump(4)

                def emit_a2a(ci, bounce):
                    ex = dramp.tile([NCORES * 2 * 128, 128], BF16,
                                    tag=f"a2a{ci}", name=f"a2a{ci}")
                    nc.gpsimd.collective_compute(
                        "AllToAll", mybir.AluOpType.bypass,
                        replica_groups=[list(range(NCORES))],
                        ins=[bounce.opt()], outs=[ex.opt()])
                    bounces.append(ex)

                def enqueue_oproj(ci):
                    """Queue chunk ci's o_proj as filler closures."""
                    ex = bounces[ci]
                    ag_all = agp.tile([128, 16 * 128], BF16, tag="ag",
                                      name="ag_all")
                    nc.sync.dma_start(
                        ag_all[:].rearrange("p (t s) -> p t s", t=16),
                        ex[:].rearrange("(t p) s -> p t s", p=128))
                    o_sb = outp.tile([128, H], BF16, tag="osb", name="o_sb")
                    state = {}

                    def mk_mm(ocg, jt):
                        def run():
                            if jt == 0:
                                state[ocg] = opsp.tile([128, 512], F32,
                                                       tag="ops", name="o_ps")
                            nc.tensor.matmul(
                                state[ocg][:],
                                ag_all[:, jt * 128:(jt + 1) * 128],
                                wo_all[:, jt * H + ocg * 512:
                                       jt * H + (ocg + 1) * 512],
                                start=(jt == 0), stop=(jt == 15))
                        return run

                    def mk_copy(ocg):
                        def run():
                            nc.vector.tensor_copy(
                                o_sb[:, ocg * 512:(ocg + 1) * 512],
                                state[ocg][:])
                        return run

                    def mk_store(ocg):
                        def run():
                            nc.sync.dma_start(
                                out[ci * 128:(ci + 1) * 128,
                                    ocg * 512:(ocg + 1) * 512],
                                o_sb[:, ocg * 512:(ocg + 1) * 512])
                        return run

                    for ocg in range(H // 512):
                        for jt in range(16):
                            fillers.append(mk_mm(ocg, jt))
                        fillers.append(mk_copy(ocg))
                        fillers.append(mk_store(ocg))

                bounce_cur = None
                for qs in range(NQS):
                    ci = qs // 2
                    if qs % 2 == 0:
                        bounce_cur = dramp.tile([NCORES * 2 * 128, 128], BF16,
                                                tag=f"bn{ci}", name=f"bn{ci}")
                        for head in range(HPC):
                            attnT_cur[head] = attnp.tile(
                                [128, CHUNK], BF16, tag=f"attnT{head}",
                                name=f"attnT{head}_{qs}")
                    # consume A2As a couple of supertiles after emission so
                    # each one has fully landed before its o_proj pumps
                    if qs == 4:
                        enqueue_oproj(0)
                    elif qs == 5:
                        enqueue_oproj(1)
                    elif qs == 6:
                        enqueue_oproj(2)
                        hold[0] = 130  # keep PE work for the A2A(3) gap
                    attention_pair(qs)
                    if qs % 2 == 1:
                        # scatter this chunk's attnT into the A2A bounce:
                        # dest core d gets s cols d*128..(d+1)*128 of the chunk
                        bv = bounce_cur[:].rearrange(
                            "(d h j) s -> h j d s", d=NCORES, h=HPC)
                        for head in range(HPC):
                            nc.sync.dma_start(
                                bv[head],
                                attnT_cur[head][:].rearrange(
                                    "j (d s) -> j d s", d=NCORES))
                        emit_a2a(ci, bounce_cur)
                # tail: the held-back chunk-2 units cover the A2A(3) wait
                hold[0] = 0
                pump(len(fillers))
                enqueue_oproj(3)
                pump(len(fillers))

    nc.compile()
    return nc


def _get_nc():
    if "nc" not in _cached:
        _cached["nc"] = _build()
    return _cached["nc"]


def _prep_inputs(hidden_states, Wq, Wk, Wv, Wo, position_ids):
    x = np.asarray(hidden_states, dtype=np.float32).reshape(S, H)
    xT = np.ascontiguousarray(
        x.T.reshape(16, 128, NSC, SC).transpose(2, 1, 0, 3)
        .reshape(NSC, 128, 16 * SC)).astype(NPBF16)

    def wshuf(W):
        n = W.shape[1]
        return np.ascontiguousarray(
            W.reshape(16, 128, n).transpose(1, 0, 2).reshape(128, 16 * n)
        ).astype(NPBF16)
    Wq = np.asarray(Wq, dtype=np.float32)
    Wk = np.asarray(Wk, dtype=np.float32)
    Wv = np.asarray(Wv, dtype=np.float32)
    Wo = np.asarray(Wo, dtype=np.float32)
    pos = np.asarray(position_ids).reshape(S).astype(np.float32)

    half = D // 2
    inv_freq = 1.0 / (THETA ** (np.arange(half, dtype=np.float32) * 2.0 / D))
    freqs = inv_freq[:, None] * pos[None, :]          # [64, S]
    c64 = np.cos(freqs, dtype=np.float32)
    s64 = np.sin(freqs, dtype=np.float32)
    cosT = np.vstack([c64, c64]).astype(NPBF16)       # [128, S]
    sinT = np.vstack([-s64, s64]).astype(NPBF16)      # signed for rotate-half
    masksl = (np.tril(np.ones((128, 128), dtype=np.float32), -1)
              * -30000.0).astype(NPBF16)
    Wo_bf = wshuf(Wo)
    ident = np.eye(128, dtype=np.float32).astype(NPBF16)

    in_maps = []
    for c in range(NCORES):
        kvh = c // 2
        in_maps.append({
            "xT": xT,
            "wq": wshuf(Wq[:, c * HPC * D:(c + 1) * HPC * D]),
            "wk": wshuf(Wk[:, kvh * D:(kvh + 1) * D]),
            "wv": wshuf(Wv[:, kvh * D:(kvh + 1) * D]),
            "wo": Wo_bf,
            "cosT": cosT,
            "sinT": sinT,
            "masksl": masksl,
            "identity": ident,
        })
    return in_maps


def _run(inputs, trace=False):
    nc = _get_nc()
    in_maps = _prep_inputs(**inputs)
    res = run_bass_kernel_spmd(nc, in_maps, list(range(NCORES)), trace=trace)
    full = np.empty((S, H), dtype=np.float32)
    for c in range(NCORES):
        shard = np.asarray(res.results[c]["out"], dtype=np.float32)
        for i in range(NCH):
            full[i * CHUNK + c * 128: i * CHUNK + (c + 1) * 128, :] = \
                shard[i * 128:(i + 1) * 128, :]
    return full.reshape(B, S, H), res


def kernel(**inputs):
    full, _ = _run(inputs, trace=False)
    return full


# revision 26
# speedup vs baseline: 1.0976x; 1.0224x over previous
"""Distributed Trainium2 kernel for DeepseekV3-style GQA attention.

Problem (hardcoded): B=1, S=4096, H=2048, NQ=16 q heads, NKV=4 kv heads,
D=128, rotate-half RoPE (theta=10000) over full head dim, causal softmax,
o_proj. 8 NeuronCores, tensor-parallel over heads:

  core c: q heads {2c, 2c+1}, kv head c//2 (replicated across the pair),
  Wq/Wk/Wv column-sharded, attention computed flash-style in bf16 with
  f32 PSUM accumulation, AllGathered in 4 sequence chunks via AllToAll,
  o_proj row-blocks [128, H] per chunk interleaved into the attention
  loop as PE filler work.

v2 changes vs v1:
  - flipped PV: V is the stationary operand, pt (probs) streams 512 wide
    per head, so the attention output lands directly in the transposed
    [d, q] layout the A2A needs -- the 64 XBAR transpose DMAs (79us of
    sync-queue time) and the per-qsub norm dance are gone, and PV drops
    from 1056 N=129 matmuls to 288 N=512 ones
  - softmax denominators: pt tiles are summed on the (idle) Vector engine
    into a per-supertile bf16 accumulator; one M=1 ones-matmul per head
    reduces it across partitions, reciprocal + DMA row-broadcast feed a
    single tensor_tensor normalize per head
  - DMA traffic spread across rings: xt chunks alternate vector/sync,
    weights+cos/sin+wo ride scalar, so no single ring serializes
  - fine-grained startup: first matmul gated on 0.5MB of wq + 1MB of xt
    instead of 5MB on one ring
  - o_proj chunks enqueued earlier (qs=4/5/6) with hold for the A2A(3) gap
"""
import os
import sys

sys.path.insert(0, "/opt/trn_rl_repo")

import numpy as np
import ml_dtypes

import concourse.bass as bass
import concourse.bacc as bacc
import concourse.mybir as mybir
import concourse.tile as tile
from concourse.bass_utils import run_bass_kernel_spmd

BF16 = mybir.dt.bfloat16
F32 = mybir.dt.float32
NPBF16 = ml_dtypes.bfloat16

B, S, H = 1, 4096, 2048
NQ, NKV, D = 16, 4, 128
THETA = 10000.0
NCORES = 8
HPC = NQ // NCORES          # q heads per core = 2
SC = 512                    # projection s-chunk
NSC = S // SC               # 8
NKT = S // 128              # 32 k tiles of 128
QS = 512                    # attention q supertile
NQS = S // QS               # 8
CHUNK = 1024                # allgather s-chunk
NCH = S // CHUNK            # 4
SCALE = 1.0 / float(np.sqrt(D))

_cached = {}


def _build():
    nc = bacc.Bacc("TRN2", target_bir_lowering=False, debug=False,
                   num_devices=NCORES)

    xT = nc.declare_dram_parameter("xT", [NSC, 128, 16 * SC], BF16, isOutput=False)
    wq = nc.declare_dram_parameter("wq", [128, 16 * HPC * D], BF16, isOutput=False)
    wk = nc.declare_dram_parameter("wk", [128, 16 * D], BF16, isOutput=False)
    wv = nc.declare_dram_parameter("wv", [128, 16 * D], BF16, isOutput=False)
    wo = nc.declare_dram_parameter("wo", [128, 16 * H], BF16, isOutput=False)
    cosT = nc.declare_dram_parameter("cosT", [D, S], BF16, isOutput=False)
    sinT = nc.declare_dram_parameter("sinT", [D, S], BF16, isOutput=False)
    masksl = nc.declare_dram_parameter("masksl", [128, 128], BF16, isOutput=False)
    identity = nc.declare_dram_parameter("identity", [128, 128], BF16, isOutput=False)
    out = nc.declare_dram_parameter("out", [NCH * 128, H], BF16, isOutput=True)

    with tile.TileContext(nc) as tc:
        with (
            tc.tile_pool(name="const", bufs=1) as constp,
            tc.tile_pool(name="persist", bufs=1) as persist,
            tc.tile_pool(name="xtp", bufs=3) as xtp,
            tc.tile_pool(name="ropep", bufs=3) as ropep,
            tc.tile_pool(name="ptp", bufs=4) as ptp,
            tc.tile_pool(name="accp", bufs=2) as accp,
            tc.tile_pool(name="attnp", bufs=2) as attnp,
            tc.tile_pool(name="smallp", bufs=2) as smallp,
            tc.tile_pool(name="rbp", bufs=2) as rbp,
            tc.tile_pool(name="agp", bufs=2) as agp,
            tc.tile_pool(name="outp", bufs=1) as outp,
            tc.tile_pool(name="dram", bufs=1, space="DRAM") as dramp,
        ):
            wq_all = persist.tile([128, 16 * HPC * D], BF16, tag="wq")
            wk_all = persist.tile([128, 16 * D], BF16, tag="wk")
            wv_all = persist.tile([128, 16 * D], BF16, tag="wv")
            cos_sb = persist.tile([128, S], BF16, tag="cos")
            sin_sb = persist.tile([128, S], BF16, tag="sin")
            msl = constp.tile([128, 128], BF16, tag="msl")
            ident = constp.tile([128, 128], BF16, tag="ident")
            ones_t = constp.tile([128, 1], BF16, tag="ones")
            ones_r = constp.tile([1, 128], BF16, tag="ones_r")
            wo_all = persist.tile([128, 16 * H], BF16, tag="wo")
            nc.gpsimd.memset(ones_t[:], 1.0)
            nc.gpsimd.memset(ones_r[:], 1.0)

            def wqt(t, h):
                return wq_all[:, t * HPC * D + h * D: t * HPC * D + (h + 1) * D]

            QT = [persist.tile([128, S], BF16, tag=f"qt{h}", name=f"qt{h}")
                  for h in range(HPC)]
            KT = persist.tile([128, S], BF16, tag="kt")
            V = [persist.tile([128, D], BF16, tag=f"v{kt}", name=f"v{kt}")
                 for kt in range(NKT)]

            # tiny barrier collective: warms the CC stream during the
            # input-DMA/proj phase so the first real A2A doesn't pay it
            bar_in = dramp.tile([1, 128], F32, tag="bar_in", name="bar_in")
            bar_out = dramp.tile([1, 128], F32, tag="bar_out", name="bar_out",
                                 addr_space="Shared")
            nc.gpsimd.collective_compute(
                "AllReduce", mybir.AluOpType.add,
                replica_groups=[list(range(NCORES))],
                ins=[bar_in.opt()], outs=[bar_out.opt()])

            # startup DMAs: the first matmul's waits coalesce to ALL DMAs
            # issued before it per-ring, so keep each ring's prefix to just
            # what chunk 0 needs first (tiny wq piece on sync; xt0's first
            # t-block is the first scalar-ring DMA, issued in the proj loop
            # below). Only sync (SP) and scalar (Act) have HWDGE rings.
            nc.sync.dma_start(wq_all[:, 0:256], wq[:, 0:256])
            nc.sync.dma_start(wq_all[:, 256:2048], wq[:, 256:2048])

            # ---- phase 1: projections (chunked over s) + RoPE ----
            with (
                tc.tile_pool(name="projps", bufs=4, space="PSUM") as projps,
                tc.tile_pool(name="vps", bufs=2, space="PSUM") as vps,
            ):
                for sc in range(NSC):
                    off = sc * SC
                    ring = nc.scalar if sc % 2 == 0 else nc.sync
                    xt_a = xtp.tile([128, 8 * SC], BF16, tag="xt",
                                    name="xt_a")
                    if sc == 0:
                        # first t-block alone so the first matmul gates on
                        # only 128KB of x
                        nc.scalar.dma_start(xt_a[:, 0:SC], xT[0][:, 0:SC])
                        nc.scalar.dma_start(xt_a[:, SC:4096],
                                            xT[0][:, SC:4096])
                        nc.scalar.dma_start(msl[:], masksl[:])
                        nc.scalar.dma_start(ident[:], identity[:])
                    else:
                        ring.dma_start(xt_a[:], xT[sc][:, 0:4096])
                    xt_b = xtp.tile([128, 8 * SC], BF16, tag="xt",
                                    name="xt_b")
                    ring.dma_start(xt_b[:], xT[sc][:, 4096:8192])
                    if sc == 2:
                        # second halves of the rope tables: first needed by
                        # chunk 4, so issue them behind chunk 2's x stream
                        nc.scalar.dma_start(cos_sb[:, 2048:S],
                                            cosT[:, 2048:S])
                        nc.scalar.dma_start(sin_sb[:, 2048:S],
                                            sinT[:, 2048:S])

                    # chunk 0's remaining weights/tables ride the sync ring,
                    # issued mid-chain so the first matmuls' coalesced waits
                    # only cover the wq prefix
                    hooks = {} if sc != 0 else {
                        3: lambda: nc.sync.dma_start(wk_all[:], wk[:]),
                        5: lambda: nc.sync.dma_start(cos_sb[:, 0:2048],
                                                     cosT[:, 0:2048]),
                        7: lambda: nc.sync.dma_start(wq_all[:, 2048:4096],
                                                     wq[:, 2048:4096]),
                        9: lambda: nc.sync.dma_start(sin_sb[:, 0:2048],
                                                     sinT[:, 0:2048]),
                        11: lambda: nc.sync.dma_start(wv_all[:], wv[:]),
                    }

                    def xts(t):
                        half = xt_a if t < 8 else xt_b
                        return half[:, (t % 8) * SC:(t % 8 + 1) * SC]

                    # q/k projections -> transposed layout [d, s]
                    def project_rope(lhs_of_t, dst, use_hooks=False):
                        ps = projps.tile([128, SC], F32, tag="proj", name="ps")
                        for t in range(16):
                            nc.tensor.matmul(ps[:], lhs_of_t(t), xts(t),
                                             start=(t == 0), stop=(t == 15))
                            if use_hooks and t in hooks:
                                hooks[t]()
                        raw = ropep.tile([128, SC], BF16, tag="raw", name="raw")
                        nc.vector.tensor_copy(raw[:], ps[:])
                        sw = ropep.tile([128, SC], BF16, tag="sw", name="sw")
                        nc.sync.dma_start(sw[0:64, :], raw[64:128, :])
                        nc.sync.dma_start(sw[64:128, :], raw[0:64, :])
                        t1 = ropep.tile([128, SC], BF16, tag="t1", name="t1")
                        nc.vector.tensor_tensor(t1[:], raw[:],
                                                cos_sb[:, off:off + SC],
                                                mybir.AluOpType.mult)
                        t2 = ropep.tile([128, SC], BF16, tag="t2", name="t2")
                        nc.vector.tensor_tensor(t2[:], sw[:],
                                                sin_sb[:, off:off + SC],
                                                mybir.AluOpType.mult)
                        nc.vector.tensor_tensor(dst[:, off:off + SC], t1[:],
                                                t2[:], mybir.AluOpType.add)

                    for h in range(HPC):
                        project_rope(lambda t, h=h: wqt(t, h), QT[h],
                                     use_hooks=(sc == 0 and h == 0))
                    project_rope(lambda t: wk_all[:, t * D:(t + 1) * D], KT)

                    # v projection, flipped: wv stationary, x streams 512
                    # wide -> vT [d, s] at full PE rate, then PE-transpose
                    # each 128-block back to the [s, d] layout PV needs
                    vt_ps = vps.tile([128, SC], F32, tag="vps", name="vt_ps")
                    for t in range(16):
                        nc.tensor.matmul(vt_ps[:],
                                         wv_all[:, t * D:(t + 1) * D],
                                         xts(t), start=(t == 0),
                                         stop=(t == 15))
                    vt_sb = ropep.tile([128, SC], BF16, tag="raw",
                                       name="vt_sb")
                    nc.vector.tensor_copy(vt_sb[:], vt_ps[:])
                    for st in range(SC // 128):
                        v_tp = vps.tile([128, 128], BF16, tag="vtp",
                                        name="v_tp")
                        nc.tensor.transpose(
                            v_tp[:], vt_sb[:, st * 128:(st + 1) * 128],
                            ident[:])
                        kti = sc * (SC // 128) + st
                        nc.vector.tensor_copy(V[kti][:], v_tp[:])

            # wo (8.4MB) deliberately loads during attention, not proj:
            # the proj phase is nearly DMA-bound (xt stream), attention
            # DMA is light, and wo is first read at the qs=4 o_proj fill
            nc.scalar.dma_start(wo_all[:], wo[:])

            # ---- phases 2+3 interleaved: attention, AG, o_proj ----
            with (
                tc.tile_pool(name="stps", bufs=2, space="PSUM") as stps,
                tc.tile_pool(name="attps", bufs=1, space="PSUM") as attps,
                tc.tile_pool(name="ops", bufs=1, space="PSUM") as opsp,
                tc.tile_pool(name="dnps", bufs=1, space="PSUM") as dnps,
            ):
                attnT_cur = [None, None]
                bounces = []
                fillers = []        # pending o_proj closures (PE work units)
                late_fillers = []   # chains gated (via dnps WAR) to run late

                def pump(n):
                    for _ in range(n):
                        if not fillers:
                            return
                        fillers.pop(0)()

                def attention_pair(qs):
                    """Both heads per kt: paired score psum [h0|h1] (2
                    banks), one 2N-wide exp, then a single V-stationary
                    matmul per head streams pt 512 wide into a [d, q]
                    accumulator -- output is born transposed. Softmax
                    denominators ride a bf16 DVE accumulator + one M=1
                    ones-matmul per head at supertile end."""
                    q_off = qs * QS
                    nkt = 4 * qs + 4
                    att = [attps.tile([128, QS], F32, tag=f"att{h}",
                                      name=f"att{h}_{qs}")
                           for h in range(HPC)]
                    acc = accp.tile([128, 2 * QS], BF16, tag="acc",
                                    name=f"acc{qs}")

                    def _pv(kt, pt):
                        for h in range(HPC):
                            nc.tensor.matmul(
                                att[h][:], V[kt][:],
                                pt[:, h * QS:(h + 1) * QS],
                                start=(kt == 0), stop=(kt == nkt - 1),
                                skip_group_check=True)

                    pend = None
                    for kt in range(nkt):
                        j = kt - 4 * qs
                        q_lo = 128 * j if j > 0 else 0
                        sp = stps.tile([128, 2 * QS], F32, tag="st",
                                       name="st_pair")
                        for h in range(HPC):
                            nc.tensor.matmul(
                                sp[:, h * QS + q_lo:(h + 1) * QS],
                                KT[:, kt * 128:(kt + 1) * 128],
                                QT[h][:, q_off + q_lo:q_off + QS],
                                start=True, stop=(j < 0),
                                skip_group_check=True)
                            if j >= 0:
                                # additive causal mask on the diagonal block
                                nc.tensor.matmul(
                                    sp[:, h * QS + q_lo:h * QS + q_lo + 128],
                                    ident[:], msl[:], start=False, stop=True,
                                    skip_group_check=True)
                        pt = ptp.tile([128, 2 * QS], BF16, tag="pt", name="pt")
                        if q_lo > 0:
                            # columns below the diagonal band are fully
                            # masked; zero them so PV and the denominator
                            # accumulate see exact zeros
                            nc.vector.memset(
                                pt[:].rearrange("p (h c) -> p h c", h=2)
                                [:, :, 0:q_lo], 0.0)
                        src = sp[:].rearrange("p (h c) -> p h c",
                                              h=2)[:, :, q_lo:QS]
                        dst = pt[:].rearrange("p (h c) -> p h c",
                                              h=2)[:, :, q_lo:QS]
                        nc.scalar.activation(dst, src,
                                             mybir.ActivationFunctionType.Exp,
                                             scale=SCALE)
                        if kt == 0:
                            nc.vector.tensor_copy(acc[:], pt[:])
                        else:
                            nc.vector.tensor_tensor(acc[:], acc[:], pt[:],
                                                    mybir.AluOpType.add)
                        pump(2)
                        if pend is not None:
                            _pv(*pend)
                        pend = (kt, pt)
                    pump(2)
                    _pv(*pend)

                    # denominators + normalize into the A2A layout.
                    # dn_t's bank is reused: row 0 takes the ones-matmul
                    # column sums, then a K=1 ones-row matmul broadcasts
                    # the reciprocal back over all 128 partitions.
                    col = (qs % 2) * QS
                    for h in range(HPC):
                        dn_t = dnps.tile([128, QS], F32, tag="dn", name="dn_t")
                        nc.tensor.matmul(dn_t[0:1, :], ones_t[:],
                                         acc[:, h * QS:(h + 1) * QS],
                                         start=True, stop=True,
                                         skip_group_check=True)
                        rec_f = smallp.tile([1, QS], F32, tag="rec_f",
                                            name="rec_f")
                        nc.vector.reciprocal_approx_fast(rec_f[:],
                                                         dn_t[0:1, :])
                        rec = smallp.tile([1, QS], BF16, tag="rec", name="rec")
                        nc.vector.tensor_copy(rec[:], rec_f[:])
                        nc.tensor.matmul(dn_t[:], ones_r[:], rec[:],
                                         start=True, stop=True,
                                         skip_group_check=True)
                        rb = rbp.tile([128, QS], BF16, tag="rb", name="rb")
                        nc.vector.tensor_copy(rb[:], dn_t[:])
                        pump(2)
                        nc.vector.tensor_tensor(
                            attnT_cur[h][:, col:col + QS], att[h][:], rb[:],
                            mybir.AluOpType.mult)
                        pump(2)
                    pump(4)

                def emit_a2a(ci, bounce):
                    ex = dramp.tile([NCORES * 2 * 128, 128], BF16,
                                    tag=f"a2a{ci}", name=f"a2a{ci}")
                    nc.gpsimd.collective_compute(
                        "AllToAll", mybir.AluOpType.bypass,
                        replica_groups=[list(range(NCORES))],
                        ins=[bounce.opt()], outs=[ex.opt()])
                    bounces.append(ex)

                def enqueue_oproj(ci, late=False):
                    """Queue chunk ci's o_proj as filler closures. With
                    late=True the chains' PSUM comes from the dnps ring, so
                    each chain is WAR-gated behind the last supertile's
                    denominator use -- the scheduler cannot hoist it early,
                    which keeps real PE work in the A2A tail window."""
                    ex = bounces[ci]
                    ag_all = agp.tile([128, 16 * 128], BF16, tag="ag",
                                      name="ag_all")
                    nc.sync.dma_start(
                        ag_all[:].rearrange("p (t s) -> p t s", t=16),
                        ex[:].rearrange("(t p) s -> p t s", p=128))
                    o_sb = outp.tile([128, H], BF16, tag="osb", name="o_sb")
                    state = {}
                    dst = late_fillers if late else fillers

                    def mk_mm(ocg, jt):
                        def run():
                            if jt == 0:
                                if late:
                                    state[ocg] = dnps.tile([128, 512], F32,
                                                           tag="dn",
                                                           name="o_ps_l")
                                else:
                                    state[ocg] = opsp.tile([128, 512], F32,
                                                           tag="ops",
                                                           name="o_ps")
                            nc.tensor.matmul(
                                state[ocg][:],
                                ag_all[:, jt * 128:(jt + 1) * 128],
                                wo_all[:, jt * H + ocg * 512:
                                       jt * H + (ocg + 1) * 512],
                                start=(jt == 0), stop=(jt == 15))
                        return run

                    def mk_copy(ocg):
                        def run():
                            nc.vector.tensor_copy(
                                o_sb[:, ocg * 512:(ocg + 1) * 512],
                                state[ocg][:])
                        return run

                    def mk_store(ocg):
                        def run():
                            nc.sync.dma_start(
                                out[ci * 128:(ci + 1) * 128,
                                    ocg * 512:(ocg + 1) * 512],
                                o_sb[:, ocg * 512:(ocg + 1) * 512])
                        return run

                    for ocg in range(H // 512):
                        for jt in range(16):
                            dst.append(mk_mm(ocg, jt))
                        dst.append(mk_copy(ocg))
                        dst.append(mk_store(ocg))

                bounce_cur = None
                for qs in range(NQS):
                    ci = qs // 2
                    if qs % 2 == 0:
                        bounce_cur = dramp.tile([NCORES * 2 * 128, 128], BF16,
                                                tag=f"bn{ci}", name=f"bn{ci}")
                        for head in range(HPC):
                            attnT_cur[head] = attnp.tile(
                                [128, CHUNK], BF16, tag=f"attnT{head}",
                                name=f"attnT{head}_{qs}")
                    # consume A2As a couple of supertiles after emission so
                    # each one has fully landed before its o_proj pumps
                    if qs == 4:
                        enqueue_oproj(0)
                    elif qs == 5:
                        enqueue_oproj(1)
                    elif qs == 6:
                        # chunk 2's chains are dnps-gated: they execute
                        # after qs=7's normalize, covering the A2A(3) flight
                        enqueue_oproj(2, late=True)
                    attention_pair(qs)
                    if qs % 2 == 1:
                        # scatter this chunk's attnT into the A2A bounce:
                        # dest core d gets s cols d*128..(d+1)*128 of the chunk
                        bv = bounce_cur[:].rearrange(
                            "(d h j) s -> h j d s", d=NCORES, h=HPC)
                        for head in range(HPC):
                            nc.sync.dma_start(
                                bv[head],
                                attnT_cur[head][:].rearrange(
                                    "j (d s) -> j d s", d=NCORES))
                        emit_a2a(ci, bounce_cur)
                # tail: chunk-2's dnps-gated chains execute here, covering
                # the A2A(3) flight; chunk 3's o_proj then runs on a warm PE
                pump(len(fillers))
                for f in late_fillers:
                    f()
                enqueue_oproj(3)
                pump(len(fillers))

    nc.compile()
    return nc


def _get_nc():
    if "nc" not in _cached:
        _cached["nc"] = _build()
    return _cached["nc"]


def _prep_inputs(hidden_states, Wq, Wk, Wv, Wo, position_ids):
    x = np.asarray(hidden_states, dtype=np.float32).reshape(S, H)
    xT = np.ascontiguousarray(
        x.T.reshape(16, 128, NSC, SC).transpose(2, 1, 0, 3)
        .reshape(NSC, 128, 16 * SC)).astype(NPBF16)

    def wshuf(W):
        n = W.shape[1]
        return np.ascontiguousarray(
            W.reshape(16, 128, n).transpose(1, 0, 2).reshape(128, 16 * n)
        ).astype(NPBF16)
    Wq = np.asarray(Wq, dtype=np.float32)
    Wk = np.asarray(Wk, dtype=np.float32)
    Wv = np.asarray(Wv, dtype=np.float32)
    Wo = np.asarray(Wo, dtype=np.float32)
    pos = np.asarray(position_ids).reshape(S).astype(np.float32)

    half = D // 2
    inv_freq = 1.0 / (THETA ** (np.arange(half, dtype=np.float32) * 2.0 / D))
    freqs = inv_freq[:, None] * pos[None, :]          # [64, S]
    c64 = np.cos(freqs, dtype=np.float32)
    s64 = np.sin(freqs, dtype=np.float32)
    cosT = np.vstack([c64, c64]).astype(NPBF16)       # [128, S]
    sinT = np.vstack([-s64, s64]).astype(NPBF16)      # signed for rotate-half
    masksl = (np.tril(np.ones((128, 128), dtype=np.float32), -1)
              * -30000.0).astype(NPBF16)
    Wo_bf = wshuf(Wo)
    ident = np.eye(128, dtype=np.float32).astype(NPBF16)

    in_maps = []
    for c in range(NCORES):
        kvh = c // 2
        in_maps.append({
            "xT": xT,
            "wq": wshuf(Wq[:, c * HPC * D:(c + 1) * HPC * D]),
            "wk": wshuf(Wk[:, kvh * D:(kvh + 1) * D]),
            "wv": wshuf(Wv[:, kvh * D:(kvh + 1) * D]),
            "wo": Wo_bf,
            "cosT": cosT,
            "sinT": sinT,
            "masksl": masksl,
            "identity": ident,
        })
    return in_maps


def _run(inputs, trace=False):
    nc = _get_nc()
    in_maps = _prep_inputs(**inputs)
    res = run_bass_kernel_spmd(nc, in_maps, list(range(NCORES)), trace=trace)
    full = np.empty((S, H), dtype=np.float32)
    for c in range(NCORES):
        shard = np.asarray(res.results[c]["out"], dtype=np.float32)
        for i in range(NCH):
            full[i * CHUNK + c * 128: i * CHUNK + (c + 1) * 128, :] = \
                shard[i * 128:(i + 1) * 128, :]
    return full.reshape(B, S, H), res


def kernel(**inputs):
    full, _ = _run(inputs, trace=False)
    return full
